# revision 9
# baseline (speedup 1.0000x reference)
"""BiGRU encoder (nn_BiGRUEncoder) as an 8-core TRN2 Bass kernel.

Contract: kernel(**inputs) takes the FULL unsharded inputs from
setup_inputs() and returns the FULL [B, T-2L, 2F] output, distributing work
across 8 NeuronCores internally.

Decomposition: the hidden dim F=1024 is split across the 8 cores (128
features each). Every core runs BOTH scan directions with the full batch
B=32, computing its 384 rows of the 3F gate pre-activations per step. After
each step the transposed h chunks ([128, 32] per direction) are exchanged
with an AllGather so the next step's recurrent matmul has the full h.T.
Input projections gi = x @ Wih.T don't depend on h and are hoisted into a
prologue as one large batched matmul per direction, stored in DRAM, and
streamed per step.

Per-step layouts: batch on partitions for gate math, with both directions
stacked ([64, X]: fwd rows 0-31, bwd rows 32-63); features on partitions for
the exchanged h.T chunks. The scan stops at T-L: the last L steps of either
direction feed no output.
"""

import sys

sys.path.insert(0, "/opt/trn_rl_repo")

import os

import numpy as np

from concourse import bacc, tile, mybir
from concourse import bass_utils

F32 = mybir.dt.float32

B = 32  # batch
T = 512  # sequence length
F = 1024  # hidden/feature dim
L = 10  # trim at both ends of T
NC = 8  # cores
P = 128  # partitions / features per core
G = 3 * P  # gate rows per core
KB = F // P  # contraction blocks


def build_gru_kernel(nc, tc, with_gbias: bool, with_nbias: bool):
    """Emit the SPMD program (identical on all 8 cores)."""
    ablate = os.environ.get("K_ABLATE", "")
    TS = 1 if ablate == "prologue" else T - L  # scan steps needed
    TO = T - 2 * L  # output steps

    xt = nc.dram_tensor("xt", [F, T * B], F32, kind="ExternalInput").ap()
    xo = nc.dram_tensor("xo", [T, B, P], F32, kind="ExternalInput").ap()
    wih = nc.dram_tensor("wih", [2, KB, P, G], F32, kind="ExternalInput").ap()
    whh = nc.dram_tensor("whh", [2, KB, P, G], F32, kind="ExternalInput").ap()
    ident = nc.dram_tensor("ident", [2 * B, 2 * B], F32, kind="ExternalInput").ap()
    if with_gbias:
        gbias = nc.dram_tensor("gbias", [2, P, G], F32, kind="ExternalInput").ap()
    if with_nbias:
        nbias = nc.dram_tensor("nbias", [2 * B, P], F32, kind="ExternalInput").ap()
    outp = nc.dram_tensor("out_own", [2, TO, B, P], F32, kind="ExternalOutput").ap()

    wih_sb = nc.alloc_sbuf_tensor("wih_sb", [P, 2 * KB * G], F32)
    whh_sb = nc.alloc_sbuf_tensor("whh_sb", [P, 2 * KB * G], F32)
    hbuf = nc.alloc_sbuf_tensor("hbuf", [2 * B, 8 * P], F32)
    ident_sb = nc.alloc_sbuf_tensor("ident_sb", [2 * B, 2 * B], F32)
    if with_gbias:
        gbias_sb = nc.alloc_sbuf_tensor("gbias_sb", [P, 2 * G], F32)
    if with_nbias:
        nbias_sb = nc.alloc_sbuf_tensor("nbias_sb", [2 * B, P], F32)

    with tc.tile_pool(name="dram", bufs=1, space="DRAM") as dpool:
        gid = [dpool.tile([T * B, G + P], F32, name=f"gid{d}") for d in (0, 1)]

        # ================= prologue =================
        for d in (0, 1):
            for k in range(KB):
                off = (d * KB + k) * G
                nc.sync.dma_start(wih_sb.ap()[:, off : off + G], wih[d, k])
                nc.sync.dma_start(whh_sb.ap()[:, off : off + G], whh[d, k])
        nc.sync.dma_start(ident_sb.ap(), ident)
        if with_gbias:
            for d in (0, 1):
                nc.sync.dma_start(gbias_sb.ap()[:, d * G : (d + 1) * G], gbias[d])
        if with_nbias:
            nc.sync.dma_start(nbias_sb.ap(), nbias)
        nc.vector.memset(hbuf.ap(), 0.0)
        # residual x chunk rides along in gid cols [G:G+P] (same row order)
        xo_flat = xo.rearrange("t b p -> (t b) p")
        for d in (0, 1):
            nc.sync.dma_start(gid[d][:, G : G + P], xo_flat)

        # Bulk input projections: gi[d] = X2d @ Wih_d.T (own 384 cols), all t.
        with (
            tc.tile_pool(name="xtp", bufs=3) as xtp,
            tc.tile_pool(name="gps", bufs=4, space="PSUM") as gps,
            tc.tile_pool(name="gis", bufs=4) as gis,
        ):
            n_m = (T * B) // P
            for m in range(n_m):
                xtile = xtp.tile([P, KB * P], F32)
                nc.sync.dma_start(
                    xtile[:].rearrange("p (k m) -> p k m", k=KB),
                    xt.rearrange("(k p) n -> p k n", p=P)[
                        :, :, m * P : (m + 1) * P
                    ],
                )
                for d in (0, 1):
                    ps = gps.tile([P, G], F32)
                    for k in range(KB):
                        nc.tensor.matmul(
                            ps[:],
                            xtile[:, P * k : P * (k + 1)],
                            wih_sb.ap()[:, (d * KB + k) * G : (d * KB + k + 1) * G],
                            start=(k == 0),
                            stop=(k == KB - 1),
                        )
                    gt = gis.tile([P, G], F32)
                    if with_gbias:
                        nc.vector.tensor_add(
                            gt[:], ps[:], gbias_sb.ap()[:, d * G : (d + 1) * G]
                        )
                    else:
                        nc.scalar.copy(gt[:], ps[:])
                    nc.sync.dma_start(gid[d][m * P : (m + 1) * P, :G], gt[:])

        # ================= scan =================
        with (
            tc.tile_pool(name="gip", bufs=6) as gip,
            tc.tile_pool(name="srz", bufs=3) as srzp,
            tc.tile_pool(name="rzp", bufs=3) as rzp,
            tc.tile_pool(name="sml", bufs=3) as sml,
            tc.tile_pool(name="snd", bufs=3) as sndp,
            tc.tile_pool(name="gth", bufs=3) as gthp,
            tc.tile_pool(name="cin", bufs=3, space="DRAM") as cinp,
            tc.tile_pool(name="cout", bufs=3, space="DRAM") as coutp,
            tc.tile_pool(name="pmm", bufs=3, space="PSUM") as pmm,
            tc.tile_pool(name="ptr", bufs=2, space="PSUM") as ptr,
        ):
            gth_prev = None
            for t in range(TS):
                gi_t = gip.tile([2 * B, G + P], F32)
                for d in (0, 1):
                    idx = t if d == 0 else T - 1 - t
                    nc.sync.dma_start(
                        gi_t[d * B : (d + 1) * B, :],
                        gid[d][idx * B : (idx + 1) * B, :],
                    )
                xo_t = gi_t[:, G : G + P]

                sl = t % 8
                if t == 0:
                    # h(-1) = 0 -> gh = 0: h = (1-z)*n + x
                    zc = sml.tile([2 * B, P], F32, tag="zc")
                    nc.scalar.activation(
                        zc[:],
                        gi_t[:, P : 2 * P],
                        mybir.ActivationFunctionType.Sigmoid,
                        scale=-1.0,
                    )
                    n = sml.tile([2 * B, P], F32, tag="n")
                    nc.scalar.activation(
                        n[:],
                        gi_t[:, 2 * P : 3 * P],
                        mybir.ActivationFunctionType.Tanh,
                    )
                    u1 = sml.tile([2 * B, P], F32, tag="u1")
                    nc.vector.tensor_mul(u1[:], zc[:], n[:])
                    hn = hbuf.ap()[:, sl * P : (sl + 1) * P]
                    nc.vector.tensor_add(hn, u1[:], xo_t)
                else:
                    pp = (t - 1) % 8
                    ps = pmm.tile([2 * B, G], F32)
                    for d in (0, 1):
                        for k in range(KB):
                            nc.tensor.matmul(
                                ps[d * B : (d + 1) * B, :],
                                gth_prev[:, (d * NC + k) * B : (d * NC + k + 1) * B],
                                whh_sb.ap()[
                                    :, (d * KB + k) * G : (d * KB + k + 1) * G
                                ],
                                start=(k == 0),
                                stop=(k == KB - 1),
                                tile_position=(0, d * B),
                                skip_group_check=True,
                            )
                    s_rz = srzp.tile([2 * B, 2 * P], F32)
                    nc.vector.tensor_add(s_rz[:], gi_t[:, : 2 * P], ps[:, : 2 * P])
                    rz = rzp.tile([2 * B, 2 * P], F32)
                    nc.scalar.activation(
                        rz[:], s_rz[:], mybir.ActivationFunctionType.Sigmoid
                    )
                    zc = sml.tile([2 * B, P], F32, tag="zc")
                    nc.scalar.activation(
                        zc[:],
                        s_rz[:, P : 2 * P],
                        mybir.ActivationFunctionType.Sigmoid,
                        scale=-1.0,
                    )
                    gn = ps[:, 2 * P : 3 * P]
                    if with_nbias:
                        gnb = sml.tile([2 * B, P], F32, tag="gnb")
                        nc.vector.tensor_add(gnb[:], gn, nbias_sb.ap())
                        gn = gnb[:]
                    t1 = sml.tile([2 * B, P], F32, tag="t1")
                    nc.vector.tensor_mul(t1[:], rz[:, :P], gn)
                    t2 = sml.tile([2 * B, P], F32, tag="t2")
                    nc.vector.tensor_add(t2[:], t1[:], gi_t[:, 2 * P : 3 * P])
                    n = sml.tile([2 * B, P], F32, tag="n")
                    nc.scalar.activation(
                        n[:], t2[:], mybir.ActivationFunctionType.Tanh
                    )
                    zh = sml.tile([2 * B, P], F32, tag="zh")
                    nc.vector.tensor_mul(
                        zh[:], rz[:, P : 2 * P], hbuf.ap()[:, pp * P : (pp + 1) * P]
                    )
                    u1 = sml.tile([2 * B, P], F32, tag="u1")
                    nc.vector.tensor_mul(u1[:], zc[:], n[:])
                    u2 = sml.tile([2 * B, P], F32, tag="u2")
                    nc.vector.tensor_add(u2[:], u1[:], zh[:])
                    hn = hbuf.ap()[:, sl * P : (sl + 1) * P]
                    nc.vector.tensor_add(hn, u2[:], xo_t)

                # flush output rows in 4-step blocks (slot-aligned in the ring)
                if t >= L and (t % 4 == 3 or t == TS - 1):
                    lo = max(t - (t % 4), L)
                    nn_ = t + 1 - lo
                    s0 = lo % 8
                    for d in (0, 1):
                        nc.sync.dma_start(
                            outp[d, lo - L : t + 1 - L].rearrange("s b c -> b s c"),
                            hbuf.ap()[
                                d * B : (d + 1) * B, s0 * P : (s0 + nn_) * P
                            ].rearrange("q (s c) -> q s c", c=P),
                        )

                # --- exchange h.T chunks via AllGather (skip on final step) ---
                if t == TS - 1:
                    continue
                tp = ptr.tile([P, 2 * B], F32)
                nc.tensor.transpose(tp[:], hn, ident_sb.ap())
                snd = sndp.tile([P, 2 * B], F32)
                nc.scalar.copy(snd[:], tp[:])
                if ablate == "noexch":
                    if gth_prev is None:
                        gth = gthp.tile([P, 2 * NC * B], F32)
                        for k in range(2 * NC):
                            nc.vector.tensor_copy(
                                gth[:, k * B : (k + 1) * B], snd[:, :B]
                            )
                        gth_prev = gth
                    continue
                cin = cinp.tile([P, 2 * B], F32)
                nc.sync.dma_start(cin[:], snd[:])
                cout = coutp.tile([NC * P, 2 * B], F32, addr_space="Shared")
                nc.gpsimd.collective_compute(
                    "AllGather",
                    mybir.AluOpType.bypass,
                    replica_groups=[list(range(NC))],
                    ins=[cin.opt()],
                    outs=[cout.opt()],
                )
                # gathered h.T back to SBUF: [128, (d, k, B)] with slot k from
                # rank k's rows [128k:128k+128], cols d*B:(d+1)*B
                gth = gthp.tile([P, 2 * NC * B], F32)
                cv = cout[:].rearrange("(k p) j -> p k j", p=P)
                for d in (0, 1):
                    nc.sync.dma_start(
                        gth[:, d * NC * B : (d + 1) * NC * B].rearrange(
                            "p (k j) -> p k j", j=B
                        ),
                        cv[:, :, d * B : (d + 1) * B],
                    )
                gth_prev = gth
    return []


def patch_deferred_waits(nc, deferred):
    assert not deferred


def make_in_maps(inputs: dict, core: int, shared: dict | None = None) -> dict:
    x = np.asarray(inputs["input_x"], np.float32)[:, :, :F]  # [B, T, F]
    own = slice(core * P, (core + 1) * P)
    if shared is None:
        shared = {}

    def own_cols(w):  # [3F, F] -> W.T own cols [F, 384]
        wt = np.ascontiguousarray(np.asarray(w, np.float32).T)
        return np.concatenate(
            [wt[:, g * F + core * P : g * F + (core + 1) * P] for g in range(3)],
            axis=1,
        )

    def own_vec(v):
        v = np.asarray(v, np.float32)
        return np.concatenate(
            [v[g * F + core * P : g * F + (core + 1) * P] for g in range(3)]
        )

    if "xt" not in shared:
        # identical for every core; build once
        shared["xt"] = np.ascontiguousarray(x.transpose(2, 1, 0).reshape(F, T * B))
        shared["xtb"] = np.ascontiguousarray(x.transpose(1, 0, 2))  # [T, B, F]
    m = {
        "xt": shared["xt"],
        "xo": np.ascontiguousarray(shared["xtb"][:, :, own]),
        "wih": np.ascontiguousarray(
            np.stack(
                [own_cols(inputs["Wih_f"]).reshape(KB, P, G),
                 own_cols(inputs["Wih_b"]).reshape(KB, P, G)]
            )
        ),
        "whh": np.ascontiguousarray(
            np.stack(
                [own_cols(inputs["Whh_f"]).reshape(KB, P, G),
                 own_cols(inputs["Whh_b"]).reshape(KB, P, G)]
            )
        ),
        "ident": np.eye(2 * B, dtype=np.float32),
    }
    # gate biases: bih (all gates) + bhh (r,z only) fold into gi; bhh_n is
    # applied inside the n-gate (it is multiplied by r together with gh_n).
    gb = []
    nb = []
    for d, (bi, bh) in enumerate(
        [(inputs["bih_f"], inputs["bhh_f"]), (inputs["bih_b"], inputs["bhh_b"])]
    ):
        bio, bho = own_vec(bi), own_vec(bh)
        gv = bio.copy()
        gv[: 2 * P] += bho[: 2 * P]
        gb.append(np.broadcast_to(gv, (P, G)))
        nb.append(np.broadcast_to(bho[2 * P :], (B, P)))
    m["_gbias"] = np.ascontiguousarray(np.stack(gb))  # [2, P, G]
    m["_nbias"] = np.ascontiguousarray(np.concatenate(nb, axis=0))  # [2B, P]
    return m


_COMPILED = {}


def _get_compiled(with_gbias: bool, with_nbias: bool):
    key = (with_gbias, with_nbias, os.environ.get("K_ABLATE", ""))
    if key not in _COMPILED:
        nc = bacc.Bacc(
            "TRN2",
            target_bir_lowering=False,
            debug=False,
            enable_asserts=True,
            num_devices=NC,
        )
        with tile.TileContext(nc) as tc:
            deferred = build_gru_kernel(nc, tc, with_gbias, with_nbias)
        patch_deferred_waits(nc, deferred)
        nc.compile()
        _COMPILED[key] = nc
    return _COMPILED[key]


def kernel(**inputs) -> np.ndarray:
    shared = {}
    maps = [make_in_maps(inputs, c, shared) for c in range(NC)]
    with_gbias = any(np.any(m["_gbias"]) for m in maps)
    with_nbias = any(np.any(m["_nbias"]) for m in maps)
    in_maps = []
    for m in maps:
        gb, nb = m.pop("_gbias"), m.pop("_nbias")
        if with_gbias:
            m["gbias"] = gb
        if with_nbias:
            m["nbias"] = nb
        in_maps.append(m)

    nc = _get_compiled(with_gbias, with_nbias)
    res = bass_utils.run_bass_kernel_spmd(nc, in_maps, core_ids=list(range(NC)))

    TO = T - 2 * L
    out = np.empty((B, TO, 2 * F), np.float32)
    for c in range(NC):
        oo = np.asarray(res.results[c]["out_own"])  # [2, TO, B, P]
        out[:, :, c * P : (c + 1) * P] = oo[0].transpose(1, 0, 2)
        out[:, :, F + c * P : F + (c + 1) * P] = oo[1].transpose(1, 0, 2)
    return out


# revision 11
# speedup vs baseline: 1.2136x; 1.2136x over previous
"""BiGRU encoder (nn_BiGRUEncoder) as an 8-core TRN2 Bass kernel.

Contract: kernel(**inputs) takes the FULL unsharded inputs from
setup_inputs() and returns the FULL [B, T-2L, 2F] output, distributing work
across 8 NeuronCores internally.

Decomposition: the hidden dim F=1024 is split across the 8 cores (128
features each). Every core runs BOTH scan directions with the full batch
B=32, computing its 384 rows of the 3F gate pre-activations per step. After
each step the transposed h chunks ([128, 32] per direction) are exchanged
with an AllGather so the next step's recurrent matmul has the full h.T.
Input projections gi = x @ Wih.T don't depend on h and are hoisted into a
prologue as one large batched matmul per direction, stored in DRAM, and
streamed per step.

Per-step layouts: batch on partitions for gate math, with both directions
stacked ([64, X]: fwd rows 0-31, bwd rows 32-63); features on partitions for
the exchanged h.T chunks. The scan stops at T-L: the last L steps of either
direction feed no output.
"""

import sys

sys.path.insert(0, "/opt/trn_rl_repo")

import os

import numpy as np

from concourse import bass, bacc, tile, mybir
from concourse import bass_utils

F32 = mybir.dt.float32

B = 32  # batch
T = 512  # sequence length
F = 1024  # hidden/feature dim
L = 10  # trim at both ends of T
NC = 8  # cores
P = 128  # partitions / features per core
G = 3 * P  # gate rows per core
KB = F // P  # contraction blocks


def build_gru_kernel(nc, tc, with_gbias: bool, with_nbias: bool):
    """Emit the SPMD program (identical on all 8 cores)."""
    ablate = os.environ.get("K_ABLATE", "")
    TS = 1 if ablate == "prologue" else T - L  # scan steps needed
    TO = T - 2 * L  # output steps

    xt = nc.dram_tensor("xt", [F, T * B], F32, kind="ExternalInput").ap()
    wih = nc.dram_tensor("wih", [2, KB, P, G], F32, kind="ExternalInput").ap()
    whh = nc.dram_tensor("whh", [2, KB, P, G], F32, kind="ExternalInput").ap()
    ident = nc.dram_tensor("ident", [2 * B, 2 * B], F32, kind="ExternalInput").ap()
    identP = nc.dram_tensor("identP", [P, P], F32, kind="ExternalInput").ap()
    if with_gbias:
        gbias = nc.dram_tensor("gbias", [2, P, G], F32, kind="ExternalInput").ap()
    if with_nbias:
        nbias = nc.dram_tensor("nbias", [2 * B, P], F32, kind="ExternalInput").ap()
    outp = nc.dram_tensor("out_own", [2, TO, B, P], F32, kind="ExternalOutput").ap()

    wih_sb = nc.alloc_sbuf_tensor("wih_sb", [P, 2 * KB * G], F32)
    whh_sb = nc.alloc_sbuf_tensor("whh_sb", [P, 2 * KB * G], F32)
    hbuf = nc.alloc_sbuf_tensor("hbuf", [2 * B, 8 * P], F32)
    ident_sb = nc.alloc_sbuf_tensor("ident_sb", [2 * B, 2 * B], F32)
    identP_sb = nc.alloc_sbuf_tensor("identP_sb", [P, P], F32)
    if with_gbias:
        gbias_sb = nc.alloc_sbuf_tensor("gbias_sb", [P, 2 * G], F32)
    if with_nbias:
        nbias_sb = nc.alloc_sbuf_tensor("nbias_sb", [2 * B, P], F32)

    with tc.tile_pool(name="dram", bufs=1, space="DRAM") as dpool:
        gid = [dpool.tile([T * B, G + P], F32, name=f"gid{d}") for d in (0, 1)]

        # ================= prologue =================
        for d in (0, 1):
            for k in range(KB):
                off = (d * KB + k) * G
                nc.sync.dma_start(wih_sb.ap()[:, off : off + G], wih[d, k])
                nc.sync.dma_start(whh_sb.ap()[:, off : off + G], whh[d, k])
        nc.sync.dma_start(ident_sb.ap(), ident)
        nc.sync.dma_start(identP_sb.ap(), identP)
        if with_gbias:
            for d in (0, 1):
                nc.sync.dma_start(gbias_sb.ap()[:, d * G : (d + 1) * G], gbias[d])
        if with_nbias:
            nc.sync.dma_start(nbias_sb.ap(), nbias)
        nc.vector.memset(hbuf.ap(), 0.0)

        # Bulk input projections: gi[d] = X2d @ Wih_d.T (own 384 cols), all t.
        pidv = nc.sync.partition_id()
        with (
            tc.tile_pool(name="xtp", bufs=3) as xtp,
            tc.tile_pool(name="gps", bufs=4, space="PSUM") as gps,
            tc.tile_pool(name="gis", bufs=4) as gis,
            tc.tile_pool(name="tpp", bufs=2, space="PSUM") as tpp,
            tc.tile_pool(name="xos", bufs=3) as xos,
        ):
            n_m = (T * B) // P
            for m in range(n_m):
                xtile = xtp.tile([P, KB * P], F32)
                nc.sync.dma_start(
                    xtile[:].rearrange("p (k m) -> p k m", k=KB),
                    xt.rearrange("(k p) n -> p k n", p=P)[
                        :, :, m * P : (m + 1) * P
                    ],
                )
                # residual x chunk: transpose own f-block [128, rows] -> rows x f
                xin = xos.tile([P, P], F32, tag="xin")
                nc.sync.dma_start(
                    xin[:], xt[bass.ts(pidv, P), m * P : (m + 1) * P]
                )
                xps = tpp.tile([P, P], F32)
                nc.tensor.transpose(xps[:], xin[:], identP_sb.ap())
                xsb = xos.tile([P, P], F32)
                nc.scalar.copy(xsb[:], xps[:])
                nc.sync.dma_start(gid[0][m * P : (m + 1) * P, G : G + P], xsb[:])
                nc.sync.dma_start(gid[1][m * P : (m + 1) * P, G : G + P], xsb[:])
                for d in (0, 1):
                    ps = gps.tile([P, G], F32)
                    for k in range(KB):
                        nc.tensor.matmul(
                            ps[:],
                            xtile[:, P * k : P * (k + 1)],
                            wih_sb.ap()[:, (d * KB + k) * G : (d * KB + k + 1) * G],
                            start=(k == 0),
                            stop=(k == KB - 1),
                        )
                    gt = gis.tile([P, G], F32)
                    if with_gbias:
                        nc.vector.tensor_add(
                            gt[:], ps[:], gbias_sb.ap()[:, d * G : (d + 1) * G]
                        )
                    else:
                        nc.scalar.copy(gt[:], ps[:])
                    nc.sync.dma_start(gid[d][m * P : (m + 1) * P, :G], gt[:])

        # ================= scan =================
        with (
            tc.tile_pool(name="gip", bufs=6) as gip,
            tc.tile_pool(name="srz", bufs=3) as srzp,
            tc.tile_pool(name="rzp", bufs=3) as rzp,
            tc.tile_pool(name="sml", bufs=3) as sml,
            tc.tile_pool(name="snd", bufs=3) as sndp,
            tc.tile_pool(name="gth", bufs=3) as gthp,
            tc.tile_pool(name="cin", bufs=3, space="DRAM") as cinp,
            tc.tile_pool(name="cout", bufs=3, space="DRAM") as coutp,
            tc.tile_pool(name="pmm", bufs=3, space="PSUM") as pmm,
            tc.tile_pool(name="ptr", bufs=2, space="PSUM") as ptr,
        ):
            gth_prev = None
            for t in range(TS):
                gi_t = gip.tile([2 * B, G + P], F32)
                for d in (0, 1):
                    idx = t if d == 0 else T - 1 - t
                    nc.sync.dma_start(
                        gi_t[d * B : (d + 1) * B, :],
                        gid[d][idx * B : (idx + 1) * B, :],
                    )
                xo_t = gi_t[:, G : G + P]

                sl = t % 8
                if t == 0:
                    # h(-1) = 0 -> gh = 0: h = (1-z)*n + x
                    zc = sml.tile([2 * B, P], F32, tag="zc")
                    nc.scalar.activation(
                        zc[:],
                        gi_t[:, P : 2 * P],
                        mybir.ActivationFunctionType.Sigmoid,
                        scale=-1.0,
                    )
                    n = sml.tile([2 * B, P], F32, tag="n")
                    nc.scalar.activation(
                        n[:],
                        gi_t[:, 2 * P : 3 * P],
                        mybir.ActivationFunctionType.Tanh,
                    )
                    u1 = sml.tile([2 * B, P], F32, tag="u1")
                    nc.vector.tensor_mul(u1[:], zc[:], n[:])
                    hn = hbuf.ap()[:, sl * P : (sl + 1) * P]
                    nc.vector.tensor_add(hn, u1[:], xo_t)
                else:
                    pp = (t - 1) % 8
                    ps = pmm.tile([2 * B, G], F32)
                    for d in (0, 1):
                        for k in range(KB):
                            nc.tensor.matmul(
                                ps[d * B : (d + 1) * B, :],
                                gth_prev[:, (d * NC + k) * B : (d * NC + k + 1) * B],
                                whh_sb.ap()[
                                    :, (d * KB + k) * G : (d * KB + k + 1) * G
                                ],
                                start=(k == 0),
                                stop=(k == KB - 1),
                                tile_position=(0, d * B),
                                skip_group_check=True,
                            )
                    s_rz = srzp.tile([2 * B, 2 * P], F32)
                    nc.vector.tensor_add(s_rz[:], gi_t[:, : 2 * P], ps[:, : 2 * P])
                    rz = rzp.tile([2 * B, 2 * P], F32)
                    nc.scalar.activation(
                        rz[:], s_rz[:], mybir.ActivationFunctionType.Sigmoid
                    )
                    zc = sml.tile([2 * B, P], F32, tag="zc")
                    nc.scalar.activation(
                        zc[:],
                        s_rz[:, P : 2 * P],
                        mybir.ActivationFunctionType.Sigmoid,
                        scale=-1.0,
                    )
                    gn = ps[:, 2 * P : 3 * P]
                    if with_nbias:
                        gnb = sml.tile([2 * B, P], F32, tag="gnb")
                        nc.vector.tensor_add(gnb[:], gn, nbias_sb.ap())
                        gn = gnb[:]
                    t1 = sml.tile([2 * B, P], F32, tag="t1")
                    nc.vector.tensor_mul(t1[:], rz[:, :P], gn)
                    t2 = sml.tile([2 * B, P], F32, tag="t2")
                    nc.vector.tensor_add(t2[:], t1[:], gi_t[:, 2 * P : 3 * P])
                    n = sml.tile([2 * B, P], F32, tag="n")
                    nc.scalar.activation(
                        n[:], t2[:], mybir.ActivationFunctionType.Tanh
                    )
                    zh = sml.tile([2 * B, P], F32, tag="zh")
                    nc.vector.tensor_mul(
                        zh[:], rz[:, P : 2 * P], hbuf.ap()[:, pp * P : (pp + 1) * P]
                    )
                    u1 = sml.tile([2 * B, P], F32, tag="u1")
                    nc.vector.tensor_mul(u1[:], zc[:], n[:])
                    u2 = sml.tile([2 * B, P], F32, tag="u2")
                    nc.vector.tensor_add(u2[:], u1[:], zh[:])
                    hn = hbuf.ap()[:, sl * P : (sl + 1) * P]
                    nc.vector.tensor_add(hn, u2[:], xo_t)

                # flush output rows in 4-step blocks (slot-aligned in the ring)
                if t >= L and (t % 4 == 3 or t == TS - 1):
                    lo = max(t - (t % 4), L)
                    nn_ = t + 1 - lo
                    s0 = lo % 8
                    for d in (0, 1):
                        nc.sync.dma_start(
                            outp[d, lo - L : t + 1 - L].rearrange("s b c -> b s c"),
                            hbuf.ap()[
                                d * B : (d + 1) * B, s0 * P : (s0 + nn_) * P
                            ].rearrange("q (s c) -> q s c", c=P),
                        )

                # --- exchange h.T chunks via AllGather (skip on final step) ---
                if t == TS - 1:
                    continue
                tp = ptr.tile([P, 2 * B], F32)
                nc.tensor.transpose(tp[:], hn, ident_sb.ap())
                snd = sndp.tile([P, 2 * B], F32)
                nc.scalar.copy(snd[:], tp[:])
                if ablate == "noexch":
                    if gth_prev is None:
                        gth = gthp.tile([P, 2 * NC * B], F32)
                        for k in range(2 * NC):
                            nc.vector.tensor_copy(
                                gth[:, k * B : (k + 1) * B], snd[:, :B]
                            )
                        gth_prev = gth
                    continue
                cin = cinp.tile([P, 2 * B], F32)
                nc.sync.dma_start(cin[:], snd[:])
                cout = coutp.tile([NC * P, 2 * B], F32, addr_space="Shared")
                nc.gpsimd.collective_compute(
                    "AllGather",
                    mybir.AluOpType.bypass,
                    replica_groups=[list(range(NC))],
                    ins=[cin.opt()],
                    outs=[cout.opt()],
                )
                # gathered h.T back to SBUF: [128, (d, k, B)] with slot k from
                # rank k's rows [128k:128k+128], cols d*B:(d+1)*B
                gth = gthp.tile([P, 2 * NC * B], F32)
                cv = cout[:].rearrange("(k p) j -> p k j", p=P)
                for d in (0, 1):
                    nc.sync.dma_start(
                        gth[:, d * NC * B : (d + 1) * NC * B].rearrange(
                            "p (k j) -> p k j", j=B
                        ),
                        cv[:, :, d * B : (d + 1) * B],
                    )
                gth_prev = gth
    return []


def patch_deferred_waits(nc, deferred):
    assert not deferred


def make_in_maps(inputs: dict, core: int, shared: dict | None = None) -> dict:
    x = np.asarray(inputs["input_x"], np.float32)[:, :, :F]  # [B, T, F]
    own = slice(core * P, (core + 1) * P)
    if shared is None:
        shared = {}

    def own_cols(w):  # [3F, F] -> W.T own cols [F, 384]
        wt = np.ascontiguousarray(np.asarray(w, np.float32).T)
        return np.concatenate(
            [wt[:, g * F + core * P : g * F + (core + 1) * P] for g in range(3)],
            axis=1,
        )

    def own_vec(v):
        v = np.asarray(v, np.float32)
        return np.concatenate(
            [v[g * F + core * P : g * F + (core + 1) * P] for g in range(3)]
        )

    if "xt" not in shared:
        # identical for every core; build once
        shared["xt"] = np.ascontiguousarray(x.transpose(2, 1, 0).reshape(F, T * B))
        shared["xtb"] = np.ascontiguousarray(x.transpose(1, 0, 2))  # [T, B, F]
    m = {
        "xt": shared["xt"],
        "wih": np.ascontiguousarray(
            np.stack(
                [own_cols(inputs["Wih_f"]).reshape(KB, P, G),
                 own_cols(inputs["Wih_b"]).reshape(KB, P, G)]
            )
        ),
        "whh": np.ascontiguousarray(
            np.stack(
                [own_cols(inputs["Whh_f"]).reshape(KB, P, G),
                 own_cols(inputs["Whh_b"]).reshape(KB, P, G)]
            )
        ),
        "ident": np.eye(2 * B, dtype=np.float32),
        "identP": np.eye(P, dtype=np.float32),
    }
    # gate biases: bih (all gates) + bhh (r,z only) fold into gi; bhh_n is
    # applied inside the n-gate (it is multiplied by r together with gh_n).
    gb = []
    nb = []
    for d, (bi, bh) in enumerate(
        [(inputs["bih_f"], inputs["bhh_f"]), (inputs["bih_b"], inputs["bhh_b"])]
    ):
        bio, bho = own_vec(bi), own_vec(bh)
        gv = bio.copy()
        gv[: 2 * P] += bho[: 2 * P]
        gb.append(np.broadcast_to(gv, (P, G)))
        nb.append(np.broadcast_to(bho[2 * P :], (B, P)))
    m["_gbias"] = np.ascontiguousarray(np.stack(gb))  # [2, P, G]
    m["_nbias"] = np.ascontiguousarray(np.concatenate(nb, axis=0))  # [2B, P]
    return m


_COMPILED = {}


def _get_compiled(with_gbias: bool, with_nbias: bool):
    key = (with_gbias, with_nbias, os.environ.get("K_ABLATE", ""))
    if key not in _COMPILED:
        nc = bacc.Bacc(
            "TRN2",
            target_bir_lowering=False,
            debug=False,
            enable_asserts=True,
            num_devices=NC,
        )
        with tile.TileContext(nc) as tc:
            deferred = build_gru_kernel(nc, tc, with_gbias, with_nbias)
        patch_deferred_waits(nc, deferred)
        nc.compile()
        _COMPILED[key] = nc
    return _COMPILED[key]


def kernel(**inputs) -> np.ndarray:
    shared = {}
    maps = [make_in_maps(inputs, c, shared) for c in range(NC)]
    with_gbias = any(np.any(m["_gbias"]) for m in maps)
    with_nbias = any(np.any(m["_nbias"]) for m in maps)
    in_maps = []
    for m in maps:
        gb, nb = m.pop("_gbias"), m.pop("_nbias")
        if with_gbias:
            m["gbias"] = gb
        if with_nbias:
            m["nbias"] = nb
        in_maps.append(m)

    nc = _get_compiled(with_gbias, with_nbias)
    res = bass_utils.run_bass_kernel_spmd(nc, in_maps, core_ids=list(range(NC)))

    TO = T - 2 * L
    out = np.empty((B, TO, 2 * F), np.float32)
    for c in range(NC):
        oo = np.asarray(res.results[c]["out_own"])  # [2, TO, B, P]
        out[:, :, c * P : (c + 1) * P] = oo[0].transpose(1, 0, 2)
        out[:, :, F + c * P : F + (c + 1) * P] = oo[1].transpose(1, 0, 2)
    return out


# revision 14
# speedup vs baseline: 2.1723x; 1.7901x over previous
"""BiGRU encoder (nn_BiGRUEncoder) as an 8-core TRN2 Bass kernel.

Contract: kernel(**inputs) takes the FULL unsharded inputs from
setup_inputs() and returns the FULL [B, T-2L, 2F] output, distributing work
across 8 NeuronCores internally.

Decomposition: the hidden dim F=1024 is split across the 8 cores (128
features each). Every core runs BOTH scan directions with the full batch
B=32, computing its 384 rows of the 3F gate pre-activations per step. After
each step the transposed h chunks ([128, 32] per direction) are exchanged
with an AllGather so the next step's recurrent matmul has the full h.T.
Input projections gi = x @ Wih.T don't depend on h and are hoisted into a
prologue as one large batched matmul per direction, stored in DRAM, and
streamed per step.

Per-step layouts: batch on partitions for gate math, with both directions
stacked ([64, X]: fwd rows 0-31, bwd rows 32-63); features on partitions for
the exchanged h.T chunks. The scan stops at T-L: the last L steps of either
direction feed no output.
"""

import sys

sys.path.insert(0, "/opt/trn_rl_repo")

import os

import numpy as np

from concourse import bass, bacc, tile, mybir
from concourse import bass_utils

F32 = mybir.dt.float32

B = 32  # batch
T = 512  # sequence length
F = 1024  # hidden/feature dim
L = 10  # trim at both ends of T
NC = 8  # cores
P = 128  # partitions / features per core
G = 3 * P  # gate rows per core
KB = F // P  # contraction blocks


def build_gru_kernel(nc, tc, with_gbias: bool, with_nbias: bool):
    """Emit the SPMD program (identical on all 8 cores)."""
    ablate = os.environ.get("K_ABLATE", "")
    TS = 1 if ablate == "prologue" else T - L  # scan steps needed
    TO = T - 2 * L  # output steps

    TB8 = T * B // NC
    xt = nc.dram_tensor("xt", [F, TB8], F32, kind="ExternalInput").ap()
    wih = nc.dram_tensor("wih", [2, KB, P, G], F32, kind="ExternalInput").ap()
    whh = nc.dram_tensor("whh", [2, KB, P, G], F32, kind="ExternalInput").ap()
    ident = nc.dram_tensor("ident", [2 * B, 2 * B], F32, kind="ExternalInput").ap()
    identP = nc.dram_tensor("identP", [P, P], F32, kind="ExternalInput").ap()
    if with_gbias:
        gbias = nc.dram_tensor("gbias", [2, P, G], F32, kind="ExternalInput").ap()
    if with_nbias:
        nbias = nc.dram_tensor("nbias", [2 * B, P], F32, kind="ExternalInput").ap()
    outp = nc.dram_tensor("out_own", [2, TO, B, P], F32, kind="ExternalOutput").ap()

    whh_sb = nc.alloc_sbuf_tensor("whh_sb", [P, 2 * KB * G], F32)
    hbuf = nc.alloc_sbuf_tensor("hbuf", [2 * B, 8 * P], F32)
    ident_sb = nc.alloc_sbuf_tensor("ident_sb", [2 * B, 2 * B], F32)
    identP_sb = nc.alloc_sbuf_tensor("identP_sb", [P, P], F32)
    if with_gbias:
        gbias_sb = nc.alloc_sbuf_tensor("gbias_sb", [P, 2 * G], F32)
    if with_nbias:
        nbias_sb = nc.alloc_sbuf_tensor("nbias_sb", [2 * B, P], F32)

    if True:
        # ================= prologue =================
        for d in (0, 1):
            for k in range(KB):
                off = (d * KB + k) * G
                nc.sync.dma_start(whh_sb.ap()[:, off : off + G], whh[d, k])
        nc.sync.dma_start(ident_sb.ap(), ident)
        nc.sync.dma_start(identP_sb.ap(), identP)
        if with_gbias:
            for d in (0, 1):
                nc.sync.dma_start(gbias_sb.ap()[:, d * G : (d + 1) * G], gbias[d])
        if with_nbias:
            nc.sync.dma_start(nbias_sb.ap(), nbias)
        nc.vector.memset(hbuf.ap(), 0.0)

        # Bulk input projections, T-sliced: this core computes gi for ALL
        # cores' gate columns over its own T/8 slice, then an AllToAll gives
        # every core its own 384 columns for all T. Wih is shipped own-cols
        # and AllGathered to full on device (cuts H2D 8x).
        pidv = nc.sync.partition_id()
        with tc.tile_pool(name="wag", bufs=1, space="DRAM") as wag:
            wihf = [
                wag.tile([NC * KB * P, G], F32, name=f"wihf{d}", addr_space="Shared")
                for d in (0, 1)
            ]
            win = wag.tile([KB * P, G], F32, name="win")
            for d in (0, 1):
                nc.sync.dma_start(
                    win[:], wih[d].rearrange("k p g -> (k p) g")
                )
                nc.gpsimd.collective_compute(
                    "AllGather",
                    mybir.AluOpType.bypass,
                    replica_groups=[list(range(NC))],
                    ins=[win.opt()],
                    outs=[wihf[d].opt()],
                )
            # wihf[d] rows: (src_core r, k, p) -> Wih_d.T[128k:128k+128, r's 384]
            a2a_in = [
                wag.tile([NC * TB8, G + (P if d == 0 else 0)], F32, name=f"a2ain{d}")
                for d in (0, 1)
            ]
            a2a_out = [
                wag.tile(
                    [NC * TB8, G + (P if d == 0 else 0)],
                    F32,
                    name=f"a2aout{d}",
                )
                for d in (0, 1)
            ]
            with (
                tc.tile_pool(name="xtp", bufs=3) as xtp,
                tc.tile_pool(name="wfp", bufs=3) as wfp,
                tc.tile_pool(name="gps", bufs=4, space="PSUM") as gps,
                tc.tile_pool(name="gis", bufs=4) as gis,
                tc.tile_pool(name="tpp", bufs=2, space="PSUM") as tpp,
                tc.tile_pool(name="xos", bufs=3) as xos,
            ):
                n_m = TB8 // P  # 16 m-tiles over this core's T-slice
                for m in range(n_m):
                    xtile = xtp.tile([P, KB * P], F32)
                    nc.sync.dma_start(
                        xtile[:].rearrange("p (k m) -> p k m", k=KB),
                        xt.rearrange("(k p) n -> p k n", p=P)[
                            :, :, m * P : (m + 1) * P
                        ],
                    )
                    # x.T blocks for the residual: all 8 f-chunks transposed
                    for r in range(NC):
                        xps = tpp.tile([P, P], F32)
                        nc.tensor.transpose(
                            xps[:], xtile[:, P * r : P * (r + 1)], identP_sb.ap()
                        )
                        xsb = xos.tile([P, P], F32, tag="xsb")
                        nc.scalar.copy(xsb[:], xps[:])
                        nc.sync.dma_start(
                            a2a_in[0][
                                r * TB8 + m * P : r * TB8 + (m + 1) * P, G : G + P
                            ],
                            xsb[:],
                        )
                    for d in (0, 1):
                        for r in range(NC):
                            wtile = wfp.tile([P, KB * G], F32, tag="wtile")
                            nc.sync.dma_start(
                                wtile[:].rearrange("p (k g) -> p k g", k=KB),
                                wihf[d][r * KB * P : (r + 1) * KB * P, :].rearrange(
                                    "(k p) g -> p k g", p=P
                                ),
                            )
                            ps = gps.tile([P, G], F32)
                            for k in range(KB):
                                nc.tensor.matmul(
                                    ps[:],
                                    xtile[:, P * k : P * (k + 1)],
                                    wtile[:, k * G : (k + 1) * G],
                                    start=(k == 0),
                                    stop=(k == KB - 1),
                                )
                            gt = gis.tile([P, G], F32)
                            if with_gbias:
                                nc.vector.tensor_add(
                                    gt[:], ps[:], gbias_sb.ap()[:, d * G : (d + 1) * G]
                                )
                            else:
                                nc.scalar.copy(gt[:], ps[:])
                            nc.sync.dma_start(
                                a2a_in[d][
                                    r * TB8 + m * P : r * TB8 + (m + 1) * P, :G
                                ],
                                gt[:],
                            )
            for d in (0, 1):
                nc.gpsimd.collective_compute(
                    "AllToAll",
                    mybir.AluOpType.bypass,
                    replica_groups=[list(range(NC))],
                    ins=[a2a_in[d].opt()],
                    outs=[a2a_out[d].opt()],
                )
            # after A2A, shard s of a2a_out[d] holds rows for t in
            # [s*T/8, (s+1)*T/8) x B, own 384 cols (+x for d=0) -> global
            # t-major order, i.e. exactly gid[d].
            gid = a2a_out

        # ================= scan =================
        with (
            tc.tile_pool(name="gip", bufs=6) as gip,
            tc.tile_pool(name="srz", bufs=3) as srzp,
            tc.tile_pool(name="rzp", bufs=3) as rzp,
            tc.tile_pool(name="sml", bufs=3) as sml,
            tc.tile_pool(name="snd", bufs=3) as sndp,
            tc.tile_pool(name="gth", bufs=3) as gthp,
            tc.tile_pool(name="cin", bufs=3, space="DRAM") as cinp,
            tc.tile_pool(name="cout", bufs=3, space="DRAM") as coutp,
            tc.tile_pool(name="pmm", bufs=3, space="PSUM") as pmm,
            tc.tile_pool(name="ptr", bufs=2, space="PSUM") as ptr,
        ):
            gth_prev = None
            for t in range(TS):
                gi_t = gip.tile([2 * B, G + P], F32)
                nc.sync.dma_start(
                    gi_t[:B, :], gid[0][t * B : (t + 1) * B, :]
                )
                idx = T - 1 - t
                nc.sync.dma_start(
                    gi_t[B:, :G], gid[1][idx * B : (idx + 1) * B, :]
                )
                nc.sync.dma_start(
                    gi_t[B:, G : G + P],
                    gid[0][idx * B : (idx + 1) * B, G : G + P],
                )
                xo_t = gi_t[:, G : G + P]

                sl = t % 8
                if t == 0:
                    # h(-1) = 0 -> gh = 0: h = (1-z)*n + x
                    zc = sml.tile([2 * B, P], F32, tag="zc")
                    nc.scalar.activation(
                        zc[:],
                        gi_t[:, P : 2 * P],
                        mybir.ActivationFunctionType.Sigmoid,
                        scale=-1.0,
                    )
                    n = sml.tile([2 * B, P], F32, tag="n")
                    nc.scalar.activation(
                        n[:],
                        gi_t[:, 2 * P : 3 * P],
                        mybir.ActivationFunctionType.Tanh,
                    )
                    u1 = sml.tile([2 * B, P], F32, tag="u1")
                    nc.vector.tensor_mul(u1[:], zc[:], n[:])
                    hn = hbuf.ap()[:, sl * P : (sl + 1) * P]
                    nc.vector.tensor_add(hn, u1[:], xo_t)
                else:
                    pp = (t - 1) % 8
                    ps = pmm.tile([2 * B, G], F32)
                    for d in (0, 1):
                        for k in range(KB):
                            nc.tensor.matmul(
                                ps[d * B : (d + 1) * B, :],
                                gth_prev[:, (d * NC + k) * B : (d * NC + k + 1) * B],
                                whh_sb.ap()[
                                    :, (d * KB + k) * G : (d * KB + k + 1) * G
                                ],
                                start=(k == 0),
                                stop=(k == KB - 1),
                                tile_position=(0, d * B),
                                skip_group_check=True,
                            )
                    s_rz = srzp.tile([2 * B, 2 * P], F32)
                    nc.vector.tensor_add(s_rz[:], gi_t[:, : 2 * P], ps[:, : 2 * P])
                    rz = rzp.tile([2 * B, 2 * P], F32)
                    nc.scalar.activation(
                        rz[:], s_rz[:], mybir.ActivationFunctionType.Sigmoid
                    )
                    zc = sml.tile([2 * B, P], F32, tag="zc")
                    nc.scalar.activation(
                        zc[:],
                        s_rz[:, P : 2 * P],
                        mybir.ActivationFunctionType.Sigmoid,
                        scale=-1.0,
                    )
                    gn = ps[:, 2 * P : 3 * P]
                    if with_nbias:
                        gnb = sml.tile([2 * B, P], F32, tag="gnb")
                        nc.vector.tensor_add(gnb[:], gn, nbias_sb.ap())
                        gn = gnb[:]
                    t1 = sml.tile([2 * B, P], F32, tag="t1")
                    nc.vector.tensor_mul(t1[:], rz[:, :P], gn)
                    t2 = sml.tile([2 * B, P], F32, tag="t2")
                    nc.vector.tensor_add(t2[:], t1[:], gi_t[:, 2 * P : 3 * P])
                    n = sml.tile([2 * B, P], F32, tag="n")
                    nc.scalar.activation(
                        n[:], t2[:], mybir.ActivationFunctionType.Tanh
                    )
                    zh = sml.tile([2 * B, P], F32, tag="zh")
                    nc.vector.tensor_mul(
                        zh[:], rz[:, P : 2 * P], hbuf.ap()[:, pp * P : (pp + 1) * P]
                    )
                    u1 = sml.tile([2 * B, P], F32, tag="u1")
                    nc.vector.tensor_mul(u1[:], zc[:], n[:])
                    u2 = sml.tile([2 * B, P], F32, tag="u2")
                    nc.vector.tensor_add(u2[:], u1[:], zh[:])
                    hn = hbuf.ap()[:, sl * P : (sl + 1) * P]
                    nc.vector.tensor_add(hn, u2[:], xo_t)

                # flush output rows in 4-step blocks (slot-aligned in the ring)
                if t >= L and (t % 4 == 3 or t == TS - 1):
                    lo = max(t - (t % 4), L)
                    nn_ = t + 1 - lo
                    s0 = lo % 8
                    for d in (0, 1):
                        nc.sync.dma_start(
                            outp[d, lo - L : t + 1 - L].rearrange("s b c -> b s c"),
                            hbuf.ap()[
                                d * B : (d + 1) * B, s0 * P : (s0 + nn_) * P
                            ].rearrange("q (s c) -> q s c", c=P),
                        )

                # --- exchange h.T chunks via AllGather (skip on final step) ---
                if t == TS - 1:
                    continue
                tp = ptr.tile([P, 2 * B], F32)
                nc.tensor.transpose(tp[:], hn, ident_sb.ap())
                snd = sndp.tile([P, 2 * B], F32)
                nc.scalar.copy(snd[:], tp[:])
                if ablate == "noexch":
                    if gth_prev is None:
                        gth = gthp.tile([P, 2 * NC * B], F32)
                        for k in range(2 * NC):
                            nc.vector.tensor_copy(
                                gth[:, k * B : (k + 1) * B], snd[:, :B]
                            )
                        gth_prev = gth
                    continue
                cin = cinp.tile([P, 2 * B], F32)
                nc.sync.dma_start(cin[:], snd[:])
                cout = coutp.tile([NC * P, 2 * B], F32, addr_space="Shared")
                nc.gpsimd.collective_compute(
                    "AllGather",
                    mybir.AluOpType.bypass,
                    replica_groups=[list(range(NC))],
                    ins=[cin.opt()],
                    outs=[cout.opt()],
                )
                # gathered h.T back to SBUF: [128, (d, k, B)] with slot k from
                # rank k's rows [128k:128k+128], cols d*B:(d+1)*B
                gth = gthp.tile([P, 2 * NC * B], F32)
                cv = cout[:].rearrange("(k p) j -> p k j", p=P)
                for d in (0, 1):
                    nc.sync.dma_start(
                        gth[:, d * NC * B : (d + 1) * NC * B].rearrange(
                            "p (k j) -> p k j", j=B
                        ),
                        cv[:, :, d * B : (d + 1) * B],
                    )
                gth_prev = gth
    return []


def patch_deferred_waits(nc, deferred):
    assert not deferred


def make_in_maps(inputs: dict, core: int, shared: dict | None = None) -> dict:
    x = np.asarray(inputs["input_x"], np.float32)[:, :, :F]  # [B, T, F]
    own = slice(core * P, (core + 1) * P)
    if shared is None:
        shared = {}

    def own_cols(w):  # [3F, F] -> W.T own cols [F, 384]
        wt = np.ascontiguousarray(np.asarray(w, np.float32).T)
        return np.concatenate(
            [wt[:, g * F + core * P : g * F + (core + 1) * P] for g in range(3)],
            axis=1,
        )

    def own_vec(v):
        v = np.asarray(v, np.float32)
        return np.concatenate(
            [v[g * F + core * P : g * F + (core + 1) * P] for g in range(3)]
        )

    if "xt" not in shared:
        # x.T in t-major column order; each core ships only its T/8 slice
        shared["xt"] = np.ascontiguousarray(x.transpose(2, 1, 0).reshape(F, T * B))
    TB8 = T * B // NC
    m = {
        "xt": np.ascontiguousarray(shared["xt"][:, core * TB8 : (core + 1) * TB8]),
        "wih": np.ascontiguousarray(
            np.stack(
                [own_cols(inputs["Wih_f"]).reshape(KB, P, G),
                 own_cols(inputs["Wih_b"]).reshape(KB, P, G)]
            )
        ),
        "whh": np.ascontiguousarray(
            np.stack(
                [own_cols(inputs["Whh_f"]).reshape(KB, P, G),
                 own_cols(inputs["Whh_b"]).reshape(KB, P, G)]
            )
        ),
        "ident": np.eye(2 * B, dtype=np.float32),
        "identP": np.eye(P, dtype=np.float32),
    }
    # gate biases: bih (all gates) + bhh (r,z only) fold into gi; bhh_n is
    # applied inside the n-gate (it is multiplied by r together with gh_n).
    gb = []
    nb = []
    for d, (bi, bh) in enumerate(
        [(inputs["bih_f"], inputs["bhh_f"]), (inputs["bih_b"], inputs["bhh_b"])]
    ):
        bio, bho = own_vec(bi), own_vec(bh)
        gv = bio.copy()
        gv[: 2 * P] += bho[: 2 * P]
        gb.append(np.broadcast_to(gv, (P, G)))
        nb.append(np.broadcast_to(bho[2 * P :], (B, P)))
    m["_gbias"] = np.ascontiguousarray(np.stack(gb))  # [2, P, G]
    m["_nbias"] = np.ascontiguousarray(np.concatenate(nb, axis=0))  # [2B, P]
    return m


_COMPILED = {}


def _get_compiled(with_gbias: bool, with_nbias: bool):
    key = (with_gbias, with_nbias, os.environ.get("K_ABLATE", ""))
    if key not in _COMPILED:
        nc = bacc.Bacc(
            "TRN2",
            target_bir_lowering=False,
            debug=False,
            enable_asserts=True,
            num_devices=NC,
        )
        with tile.TileContext(nc) as tc:
            deferred = build_gru_kernel(nc, tc, with_gbias, with_nbias)
        patch_deferred_waits(nc, deferred)
        nc.compile()
        _COMPILED[key] = nc
    return _COMPILED[key]


def kernel(**inputs) -> np.ndarray:
    shared = {}
    maps = [make_in_maps(inputs, c, shared) for c in range(NC)]
    with_gbias = any(np.any(m["_gbias"]) for m in maps)
    with_nbias = any(np.any(m["_nbias"]) for m in maps)
    in_maps = []
    for m in maps:
        gb, nb = m.pop("_gbias"), m.pop("_nbias")
        if with_gbias:
            m["gbias"] = gb
        if with_nbias:
            m["nbias"] = nb
        in_maps.append(m)

    nc = _get_compiled(with_gbias, with_nbias)
    res = bass_utils.run_bass_kernel_spmd(nc, in_maps, core_ids=list(range(NC)))

    TO = T - 2 * L
    out = np.empty((B, TO, 2 * F), np.float32)
    for c in range(NC):
        oo = np.asarray(res.results[c]["out_own"])  # [2, TO, B, P]
        out[:, :, c * P : (c + 1) * P] = oo[0].transpose(1, 0, 2)
        out[:, :, F + c * P : F + (c + 1) * P] = oo[1].transpose(1, 0, 2)
    return out


# revision 16
# speedup vs baseline: 2.1819x; 1.0044x over previous
"""BiGRU encoder (nn_BiGRUEncoder) as an 8-core TRN2 Bass kernel.

Contract: kernel(**inputs) takes the FULL unsharded inputs from
setup_inputs() and returns the FULL [B, T-2L, 2F] output, distributing work
across 8 NeuronCores internally.

Decomposition: the hidden dim F=1024 is split across the 8 cores (128
features each). Every core runs BOTH scan directions with the full batch
B=32, computing its 384 rows of the 3F gate pre-activations per step. After
each step the transposed h chunks ([128, 32] per direction) are exchanged
with an AllGather so the next step's recurrent matmul has the full h.T.
Input projections gi = x @ Wih.T don't depend on h and are hoisted into a
prologue as one large batched matmul per direction, stored in DRAM, and
streamed per step.

Per-step layouts: batch on partitions for gate math, with both directions
stacked ([64, X]: fwd rows 0-31, bwd rows 32-63); features on partitions for
the exchanged h.T chunks. The scan stops at T-L: the last L steps of either
direction feed no output.
"""

import sys

sys.path.insert(0, "/opt/trn_rl_repo")

import os

import numpy as np

from concourse import bass, bacc, tile, mybir
from concourse import bass_utils

F32 = mybir.dt.float32

B = 32  # batch
T = 512  # sequence length
F = 1024  # hidden/feature dim
L = 10  # trim at both ends of T
NC = 8  # cores
P = 128  # partitions / features per core
G = 3 * P  # gate rows per core
KB = F // P  # contraction blocks


def build_gru_kernel(nc, tc, with_gbias: bool, with_nbias: bool):
    """Emit the SPMD program (identical on all 8 cores)."""
    ablate = os.environ.get("K_ABLATE", "")
    TS = 1 if ablate == "prologue" else T - L  # scan steps needed
    TO = T - 2 * L  # output steps

    TB8 = T * B // NC
    xt = nc.dram_tensor("xt", [F, TB8], F32, kind="ExternalInput").ap()
    wih = nc.dram_tensor("wih", [2, KB, P, G], F32, kind="ExternalInput").ap()
    whh = nc.dram_tensor("whh", [2, KB, P, G], F32, kind="ExternalInput").ap()
    ident = nc.dram_tensor("ident", [2 * B, 2 * B], F32, kind="ExternalInput").ap()
    identP = nc.dram_tensor("identP", [P, P], F32, kind="ExternalInput").ap()
    if with_gbias:
        gbias = nc.dram_tensor("gbias", [2, P, G], F32, kind="ExternalInput").ap()
    if with_nbias:
        nbias = nc.dram_tensor("nbias", [2 * B, P], F32, kind="ExternalInput").ap()
    outp = nc.dram_tensor("out_own", [2, TO, B, P], F32, kind="ExternalOutput").ap()

    whh_sb = nc.alloc_sbuf_tensor("whh_sb", [P, 2 * KB * G], F32)
    hbuf = nc.alloc_sbuf_tensor("hbuf", [2 * B, 8 * P], F32)
    ident_sb = nc.alloc_sbuf_tensor("ident_sb", [2 * B, 2 * B], F32)
    identP_sb = nc.alloc_sbuf_tensor("identP_sb", [P, P], F32)
    if with_gbias:
        gbias_sb = nc.alloc_sbuf_tensor("gbias_sb", [P, 2 * G], F32)
    if with_nbias:
        nbias_sb = nc.alloc_sbuf_tensor("nbias_sb", [2 * B, P], F32)

    if True:
        # ================= prologue =================
        for d in (0, 1):
            for k in range(KB):
                off = (d * KB + k) * G
                nc.sync.dma_start(whh_sb.ap()[:, off : off + G], whh[d, k])
        nc.sync.dma_start(ident_sb.ap(), ident)
        nc.sync.dma_start(identP_sb.ap(), identP)
        if with_gbias:
            for d in (0, 1):
                nc.sync.dma_start(gbias_sb.ap()[:, d * G : (d + 1) * G], gbias[d])
        if with_nbias:
            nc.sync.dma_start(nbias_sb.ap(), nbias)
        nc.vector.memset(hbuf.ap(), 0.0)

        # Bulk input projections, T-sliced: this core computes gi for ALL
        # cores' gate columns over its own T/8 slice, then an AllToAll gives
        # every core its own 384 columns for all T. Wih is shipped own-cols
        # and AllGathered to full on device (cuts H2D 8x).
        pidv = nc.sync.partition_id()
        with tc.tile_pool(name="wag", bufs=1, space="DRAM") as wag:
            wihf = [
                wag.tile([NC * KB * P, G], F32, name=f"wihf{d}", addr_space="Shared")
                for d in (0, 1)
            ]
            win = wag.tile([KB * P, G], F32, name="win")
            for d in (0, 1):
                nc.sync.dma_start(
                    win[:], wih[d].rearrange("k p g -> (k p) g")
                )
                nc.gpsimd.collective_compute(
                    "AllGather",
                    mybir.AluOpType.bypass,
                    replica_groups=[list(range(NC))],
                    ins=[win.opt()],
                    outs=[wihf[d].opt()],
                )
            # wihf[d] rows: (src_core r, k, p) -> Wih_d.T[128k:128k+128, r's 384]
            a2a_in = [
                wag.tile([NC * TB8, G + P], F32, name=f"a2ain{d}")
                for d in (0, 1)
            ]
            a2a_out = [
                wag.tile([NC * TB8, G + P], F32, name=f"a2aout{d}")
                for d in (0, 1)
            ]
            n_m = TB8 // P  # 16 m-tiles over this core's T-slice
            with (
                tc.tile_pool(name="xtp", bufs=3) as xtp,
                tc.tile_pool(name="wfp", bufs=1) as wfp,
                tc.tile_pool(name="gps", bufs=4, space="PSUM") as gps,
                tc.tile_pool(name="gis", bufs=4) as gis,
                tc.tile_pool(name="tpp", bufs=2, space="PSUM") as tpp,
                tc.tile_pool(name="xos", bufs=3) as xos,
            ):
                for d in (0, 1):
                    # full Wih for this direction, SBUF-resident once
                    wfull = wfp.tile([P, NC * KB * G], F32, tag="wfull")
                    nc.sync.dma_start(
                        wfull[:].rearrange("p (r k g) -> p r k g", r=NC, k=KB),
                        wihf[d][:].rearrange("(r k p) g -> p r k g", p=P, k=KB),
                    )
                    for m in range(n_m):
                        xtile = xtp.tile([P, KB * P], F32)
                        nc.sync.dma_start(
                            xtile[:].rearrange("p (k m) -> p k m", k=KB),
                            xt.rearrange("(k p) n -> p k n", p=P)[
                                :, :, m * P : (m + 1) * P
                            ],
                        )
                        if d == 0:
                            # x.T blocks for the residual: all 8 f-chunks
                            for r in range(NC):
                                xps = tpp.tile([P, P], F32)
                                nc.tensor.transpose(
                                    xps[:],
                                    xtile[:, P * r : P * (r + 1)],
                                    identP_sb.ap(),
                                )
                                xsb = xos.tile([P, P], F32, tag="xsb")
                                nc.scalar.copy(xsb[:], xps[:])
                                for dd in (0, 1):
                                    nc.sync.dma_start(
                                        a2a_in[dd][
                                            r * TB8 + m * P : r * TB8 + (m + 1) * P,
                                            G : G + P,
                                        ],
                                        xsb[:],
                                    )
                        for r in range(NC):
                            ps = gps.tile([P, G], F32)
                            for k in range(KB):
                                nc.tensor.matmul(
                                    ps[:],
                                    xtile[:, P * k : P * (k + 1)],
                                    wfull[:, (r * KB + k) * G : (r * KB + k + 1) * G],
                                    start=(k == 0),
                                    stop=(k == KB - 1),
                                )
                            gt = gis.tile([P, G], F32)
                            if with_gbias:
                                nc.vector.tensor_add(
                                    gt[:],
                                    ps[:],
                                    gbias_sb.ap()[:, d * G : (d + 1) * G],
                                )
                            else:
                                nc.scalar.copy(gt[:], ps[:])
                            nc.sync.dma_start(
                                a2a_in[d][
                                    r * TB8 + m * P : r * TB8 + (m + 1) * P, :G
                                ],
                                gt[:],
                            )
            for d in (0, 1):
                nc.gpsimd.collective_compute(
                    "AllToAll",
                    mybir.AluOpType.bypass,
                    replica_groups=[list(range(NC))],
                    ins=[a2a_in[d].opt()],
                    outs=[a2a_out[d].opt()],
                )
            # after A2A, shard s of a2a_out[d] holds rows for t in
            # [s*T/8, (s+1)*T/8) x B, own 384 cols (+x for d=0) -> global
            # t-major order, i.e. exactly gid[d].
            gid = a2a_out

        # ================= scan =================
        with (
            tc.tile_pool(name="gip", bufs=6) as gip,
            tc.tile_pool(name="srz", bufs=3) as srzp,
            tc.tile_pool(name="rzp", bufs=3) as rzp,
            tc.tile_pool(name="sml", bufs=3) as sml,
            tc.tile_pool(name="snd", bufs=3) as sndp,
            tc.tile_pool(name="gth", bufs=3) as gthp,
            tc.tile_pool(name="cin", bufs=3, space="DRAM") as cinp,
            tc.tile_pool(name="cout", bufs=3, space="DRAM") as coutp,
            tc.tile_pool(name="pmm", bufs=3, space="PSUM") as pmm,
            tc.tile_pool(name="ptr", bufs=2, space="PSUM") as ptr,
        ):
            gth_prev = None
            for t in range(TS):
                gi_t = gip.tile([2 * B, G + P], F32)
                nc.sync.dma_start(
                    gi_t[:B, :], gid[0][t * B : (t + 1) * B, :]
                )
                idx = T - 1 - t
                nc.sync.dma_start(
                    gi_t[B:, :], gid[1][idx * B : (idx + 1) * B, :]
                )
                xo_t = gi_t[:, G : G + P]

                sl = t % 8
                if t == 0:
                    # h(-1) = 0 -> gh = 0: h = (1-z)*n + x
                    zc = sml.tile([2 * B, P], F32, tag="zc")
                    nc.scalar.activation(
                        zc[:],
                        gi_t[:, P : 2 * P],
                        mybir.ActivationFunctionType.Sigmoid,
                        scale=-1.0,
                    )
                    n = sml.tile([2 * B, P], F32, tag="n")
                    nc.scalar.activation(
                        n[:],
                        gi_t[:, 2 * P : 3 * P],
                        mybir.ActivationFunctionType.Tanh,
                    )
                    u1 = sml.tile([2 * B, P], F32, tag="u1")
                    nc.vector.tensor_mul(u1[:], zc[:], n[:])
                    hn = hbuf.ap()[:, sl * P : (sl + 1) * P]
                    nc.vector.tensor_add(hn, u1[:], xo_t)
                else:
                    pp = (t - 1) % 8
                    ps = pmm.tile([2 * B, G], F32)
                    for d in (0, 1):
                        for k in range(KB):
                            nc.tensor.matmul(
                                ps[d * B : (d + 1) * B, :],
                                gth_prev[:, (d * NC + k) * B : (d * NC + k + 1) * B],
                                whh_sb.ap()[
                                    :, (d * KB + k) * G : (d * KB + k + 1) * G
                                ],
                                start=(k == 0),
                                stop=(k == KB - 1),
                                tile_position=(0, d * B),
                                skip_group_check=True,
                            )
                    s_rz = srzp.tile([2 * B, 2 * P], F32)
                    nc.vector.tensor_add(s_rz[:], gi_t[:, : 2 * P], ps[:, : 2 * P])
                    rz = rzp.tile([2 * B, 2 * P], F32)
                    nc.scalar.activation(
                        rz[:], s_rz[:], mybir.ActivationFunctionType.Sigmoid
                    )
                    zc = sml.tile([2 * B, P], F32, tag="zc")
                    nc.scalar.activation(
                        zc[:],
                        s_rz[:, P : 2 * P],
                        mybir.ActivationFunctionType.Sigmoid,
                        scale=-1.0,
                    )
                    gn = ps[:, 2 * P : 3 * P]
                    if with_nbias:
                        gnb = sml.tile([2 * B, P], F32, tag="gnb")
                        nc.vector.tensor_add(gnb[:], gn, nbias_sb.ap())
                        gn = gnb[:]
                    t1 = sml.tile([2 * B, P], F32, tag="t1")
                    nc.vector.tensor_mul(t1[:], rz[:, :P], gn)
                    t2 = sml.tile([2 * B, P], F32, tag="t2")
                    nc.vector.tensor_add(t2[:], t1[:], gi_t[:, 2 * P : 3 * P])
                    n = sml.tile([2 * B, P], F32, tag="n")
                    nc.scalar.activation(
                        n[:], t2[:], mybir.ActivationFunctionType.Tanh
                    )
                    zh = sml.tile([2 * B, P], F32, tag="zh")
                    nc.vector.tensor_mul(
                        zh[:], rz[:, P : 2 * P], hbuf.ap()[:, pp * P : (pp + 1) * P]
                    )
                    u1 = sml.tile([2 * B, P], F32, tag="u1")
                    nc.vector.tensor_mul(u1[:], zc[:], n[:])
                    u2 = sml.tile([2 * B, P], F32, tag="u2")
                    nc.vector.tensor_add(u2[:], u1[:], zh[:])
                    hn = hbuf.ap()[:, sl * P : (sl + 1) * P]
                    nc.vector.tensor_add(hn, u2[:], xo_t)

                # flush output rows in 4-step blocks (slot-aligned in the ring)
                if t >= L and (t % 4 == 3 or t == TS - 1):
                    lo = max(t - (t % 4), L)
                    nn_ = t + 1 - lo
                    s0 = lo % 8
                    for d in (0, 1):
                        nc.sync.dma_start(
                            outp[d, lo - L : t + 1 - L].rearrange("s b c -> b s c"),
                            hbuf.ap()[
                                d * B : (d + 1) * B, s0 * P : (s0 + nn_) * P
                            ].rearrange("q (s c) -> q s c", c=P),
                        )

                # --- exchange h.T chunks via AllGather (skip on final step) ---
                if t == TS - 1:
                    continue
                tp = ptr.tile([P, 2 * B], F32)
                nc.tensor.transpose(tp[:], hn, ident_sb.ap())
                snd = sndp.tile([P, 2 * B], F32)
                nc.scalar.copy(snd[:], tp[:])
                if ablate == "noexch":
                    if gth_prev is None:
                        gth = gthp.tile([P, 2 * NC * B], F32)
                        for k in range(2 * NC):
                            nc.vector.tensor_copy(
                                gth[:, k * B : (k + 1) * B], snd[:, :B]
                            )
                        gth_prev = gth
                    continue
                cin = cinp.tile([P, 2 * B], F32)
                nc.sync.dma_start(cin[:], snd[:])
                cout = coutp.tile([NC * P, 2 * B], F32, addr_space="Shared")
                nc.gpsimd.collective_compute(
                    "AllGather",
                    mybir.AluOpType.bypass,
                    replica_groups=[list(range(NC))],
                    ins=[cin.opt()],
                    outs=[cout.opt()],
                )
                # gathered h.T back to SBUF: [128, (d, k, B)] with slot k from
                # rank k's rows [128k:128k+128], cols d*B:(d+1)*B
                gth = gthp.tile([P, 2 * NC * B], F32)
                nc.sync.dma_start(
                    gth[:].rearrange("p (d k j) -> p d k j", d=2, j=B),
                    cout[:].rearrange("(k p) (d j) -> p d k j", p=P, j=B),
                )
                gth_prev = gth
    return []


def patch_deferred_waits(nc, deferred):
    assert not deferred


def make_in_maps(inputs: dict, core: int, shared: dict | None = None) -> dict:
    x = np.asarray(inputs["input_x"], np.float32)[:, :, :F]  # [B, T, F]
    own = slice(core * P, (core + 1) * P)
    if shared is None:
        shared = {}

    def own_cols(w):  # [3F, F] -> W.T own cols [F, 384]
        wt = np.ascontiguousarray(np.asarray(w, np.float32).T)
        return np.concatenate(
            [wt[:, g * F + core * P : g * F + (core + 1) * P] for g in range(3)],
            axis=1,
        )

    def own_vec(v):
        v = np.asarray(v, np.float32)
        return np.concatenate(
            [v[g * F + core * P : g * F + (core + 1) * P] for g in range(3)]
        )

    if "xt" not in shared:
        # x.T in t-major column order; each core ships only its T/8 slice
        shared["xt"] = np.ascontiguousarray(x.transpose(2, 1, 0).reshape(F, T * B))
    TB8 = T * B // NC
    m = {
        "xt": np.ascontiguousarray(shared["xt"][:, core * TB8 : (core + 1) * TB8]),
        "wih": np.ascontiguousarray(
            np.stack(
                [own_cols(inputs["Wih_f"]).reshape(KB, P, G),
                 own_cols(inputs["Wih_b"]).reshape(KB, P, G)]
            )
        ),
        "whh": np.ascontiguousarray(
            np.stack(
                [own_cols(inputs["Whh_f"]).reshape(KB, P, G),
                 own_cols(inputs["Whh_b"]).reshape(KB, P, G)]
            )
        ),
        "ident": np.eye(2 * B, dtype=np.float32),
        "identP": np.eye(P, dtype=np.float32),
    }
    # gate biases: bih (all gates) + bhh (r,z only) fold into gi; bhh_n is
    # applied inside the n-gate (it is multiplied by r together with gh_n).
    gb = []
    nb = []
    for d, (bi, bh) in enumerate(
        [(inputs["bih_f"], inputs["bhh_f"]), (inputs["bih_b"], inputs["bhh_b"])]
    ):
        bio, bho = own_vec(bi), own_vec(bh)
        gv = bio.copy()
        gv[: 2 * P] += bho[: 2 * P]
        gb.append(np.broadcast_to(gv, (P, G)))
        nb.append(np.broadcast_to(bho[2 * P :], (B, P)))
    m["_gbias"] = np.ascontiguousarray(np.stack(gb))  # [2, P, G]
    m["_nbias"] = np.ascontiguousarray(np.concatenate(nb, axis=0))  # [2B, P]
    return m


_COMPILED = {}


def _get_compiled(with_gbias: bool, with_nbias: bool):
    key = (with_gbias, with_nbias, os.environ.get("K_ABLATE", ""))
    if key not in _COMPILED:
        nc = bacc.Bacc(
            "TRN2",
            target_bir_lowering=False,
            debug=False,
            enable_asserts=True,
            num_devices=NC,
        )
        with tile.TileContext(nc) as tc:
            deferred = build_gru_kernel(nc, tc, with_gbias, with_nbias)
        patch_deferred_waits(nc, deferred)
        nc.compile()
        _COMPILED[key] = nc
    return _COMPILED[key]


def kernel(**inputs) -> np.ndarray:
    shared = {}
    maps = [make_in_maps(inputs, c, shared) for c in range(NC)]
    with_gbias = any(np.any(m["_gbias"]) for m in maps)
    with_nbias = any(np.any(m["_nbias"]) for m in maps)
    in_maps = []
    for m in maps:
        gb, nb = m.pop("_gbias"), m.pop("_nbias")
        if with_gbias:
            m["gbias"] = gb
        if with_nbias:
            m["nbias"] = nb
        in_maps.append(m)

    nc = _get_compiled(with_gbias, with_nbias)
    res = bass_utils.run_bass_kernel_spmd(nc, in_maps, core_ids=list(range(NC)))

    TO = T - 2 * L
    out = np.empty((B, TO, 2 * F), np.float32)
    for c in range(NC):
        oo = np.asarray(res.results[c]["out_own"])  # [2, TO, B, P]
        out[:, :, c * P : (c + 1) * P] = oo[0].transpose(1, 0, 2)
        out[:, :, F + c * P : F + (c + 1) * P] = oo[1].transpose(1, 0, 2)
    return out
